# revision 1
# baseline (speedup 1.0000x reference)
"""Trainium2 Bass kernel for ContextAwareArtRecSys (gnn_message_passing).

Math fold: the reference is
    score[e] = concat(z_u[src] @ Wu.T + bu, z_i[dst] @ Wi.T + bi) @ wo.T + bo
Everything after the gather is linear, so with
    vu = wo[:, :128] @ Wu          (256-vector)
    vi = wo[:, 128:] @ Wi          (256-vector)
    c  = wo[:, :128]@bu + wo[:, 128:]@bi + bo   (scalar)
we have score[e] = (z_u @ vu)[src] + (z_i @ vi)[dst] + c.

Sharding: edges are bucketed to cores BY DST RANGE (core k owns items
[k*12500, (k+1)*12500) and all edges pointing at them), sorted by dst and
packed into 17 rows x 3968 slots, whole dst-segments per row. Then:

  - item side needs NO gather and NO collective: each core expands its
    local item scores s_i across its edge slots with an indirect SCATTER
    of 12.5k values to segment-start slots followed by a masked-reset
    prefix scan (state = M*state + V) on DVE - exact, two instructions.
  - user side: compute the 6250-entry local user score shard, AllGather
    the 50k-entry table, then 17 indirect gathers (3968 descriptors each).

Indirect-DMA service is the wall (the SWDGE queue's SDMA engines process
descriptors at ~7 ns each, ~2 engines per queue), so the 21 indirect DMAs
are spread round-robin over 4 SWDGE queues (independent engine sets) and
the score tables are stored partition-major (one big descriptor per
partition instead of 12k 4-byte descriptors for a node-major transpose) -
the gather/scatter index values absorb the layout on the host.
"""

import numpy as np

N_CORES = 8
N_USERS, N_ITEMS, E, H = 50000, 100000, 500000, 256
HALF = H // 2

U_SH = N_USERS // N_CORES          # 6250 users per core
I_SH = N_ITEMS // N_CORES          # 12500 items per core

U_TILES = 50                       # padded user row-tiles (6400 rows)
I_TILES = 100                      # padded item row-tiles (12800 rows)
CHUNK_T = 10                       # row-tiles per z DMA chunk (1.25 MB)
U_CHUNKS = U_TILES // CHUNK_T      # 5
I_CHUNKS = I_TILES // CHUNK_T      # 10
U_PAD = U_TILES * 128              # 6400 stored user scores per core
I_PAD = I_TILES * 128              # 12800 stored item scores per core

N_G = 17                           # user gather instructions
GCOLS = 31                         # idx columns per gather (31*128 = 3968)
RL = GCOLS * 128                   # 3968 slots per row / per gather
E_CAP = N_G * RL                   # 67456 edge slots per core

NSC = 4                            # scatter instructions
SC_N = I_PAD // NSC                # 3200 values per scatter
SCCOLS = SC_N // 128               # 25
OOB = 1 << 20                      # scatter index for "skip this value"
N_QUEUES = 4                       # SWDGE queues for indirect DMAs

_CACHE = {}


def _build():
    if "nc" in _CACHE:
        return _CACHE["nc"]
    import concourse.bass as bass
    import concourse.tile as tile
    import concourse.mybir as mybir
    from concourse import bacc
    from concourse.bass import IndirectOffsetOnAxis

    f32 = mybir.dt.float32
    bf16 = mybir.dt.bfloat16
    i32 = mybir.dt.int32

    nc = bacc.Bacc("TRN2", target_bir_lowering=False, debug=False,
                   num_devices=N_CORES, dynamic_dma_scratch_size=32768,
                   num_swdge_queues=N_QUEUES)

    qnames = ["qPoolDynamic"] + [f"qPoolDynamic{i}" for i in range(1, N_QUEUES)]

    zu = nc.dram_tensor("zu", [128, U_TILES * H], bf16, kind="ExternalInput")
    zi = nc.dram_tensor("zi", [128, I_TILES * H], bf16, kind="ExternalInput")
    w_user = nc.dram_tensor("w_user", [HALF, H], f32, kind="ExternalInput")
    w_item = nc.dram_tensor("w_item", [HALF, H], f32, kind="ExternalInput")
    wo_u = nc.dram_tensor("wo_u", [HALF, 1], f32, kind="ExternalInput")
    wo_i = nc.dram_tensor("wo_i", [HALF, 1], f32, kind="ExternalInput")
    b_user = nc.dram_tensor("b_user", [HALF, 1], f32, kind="ExternalInput")
    b_item = nc.dram_tensor("b_item", [HALF, 1], f32, kind="ExternalInput")
    b_out = nc.dram_tensor("b_out", [1, 1], f32, kind="ExternalInput")
    idxu = nc.dram_tensor("idxu", [128, N_G * GCOLS], i32, kind="ExternalInput")
    idxsc = nc.dram_tensor("idxsc", [128, NSC * SCCOLS], i32,
                           kind="ExternalInput")
    mrow = nc.dram_tensor("mrow", [N_G, RL], f32, kind="ExternalInput")
    out = nc.dram_tensor("out", [N_G, RL], f32, kind="ExternalOutput")

    s_uc = nc.dram_tensor("s_uc", [U_PAD, 1], f32)
    s_ic = nc.dram_tensor("s_ic", [I_PAD, 1], f32)
    s_uf = nc.dram_tensor("s_uf", [N_CORES * U_PAD, 1], f32,
                          addr_space="Shared")
    dv = nc.dram_tensor("dv", [E_CAP, 1], f32)

    groups = [list(range(N_CORES))]

    with tile.TileContext(nc) as tc:
        with (
            tc.tile_pool(name="consts", bufs=1) as consts,
            tc.tile_pool(name="zpool", bufs=3) as zpool,
            tc.tile_pool(name="scpool", bufs=2) as scpool,
            tc.tile_pool(name="spool", bufs=1) as spool,
            tc.tile_pool(name="gpool", bufs=1) as gpool,
            tc.tile_pool(name="psum", bufs=2, space="PSUM") as psum,
        ):
            # ---- fold vu / vi / c on PE ----
            wu_t = consts.tile([HALF, H], f32)
            nc.sync.dma_start(wu_t[:], w_user.ap())
            wi_t = consts.tile([HALF, H], f32)
            nc.sync.dma_start(wi_t[:], w_item.ap())
            wou_t = consts.tile([HALF, 1], f32)
            nc.sync.dma_start(wou_t[:], wo_u.ap())
            woi_t = consts.tile([HALF, 1], f32)
            nc.sync.dma_start(woi_t[:], wo_i.ap())
            bu_t = consts.tile([HALF, 1], f32)
            nc.sync.dma_start(bu_t[:], b_user.ap())
            bi_t = consts.tile([HALF, 1], f32)
            nc.sync.dma_start(bi_t[:], b_item.ap())
            bo_t = consts.tile([1, 1], f32)
            nc.sync.dma_start(bo_t[:], b_out.ap())

            # all index/mask inputs up front so nothing stalls later
            idxu_t = gpool.tile([128, N_G * GCOLS], i32)
            nc.sync.dma_start(idxu_t[:], idxu.ap())
            idxsc_t = gpool.tile([128, NSC * SCCOLS], i32)
            nc.sync.dma_start(idxsc_t[:], idxsc.ap())
            m_t = gpool.tile([N_G, RL], f32)
            nc.sync.dma_start(m_t[:], mrow.ap())
            z0_t = gpool.tile([N_G, RL], f32, tag="zsc")
            nc.vector.memset(z0_t[:], 0.0)
            nc.sync.dma_start(
                dv.ap().rearrange("(a b) one -> a (b one)", a=N_G), z0_t[:]
            )

            # replicate wo halves across the free dim: rep[k, m] = wo[k]
            ones_kk = consts.tile([HALF, HALF], f32)
            nc.vector.memset(ones_kk[:], 1.0)
            wou_rep = consts.tile([HALF, HALF], f32)
            nc.vector.tensor_scalar_mul(wou_rep[:], ones_kk[:], wou_t[:])
            woi_rep = consts.tile([HALF, HALF], f32)
            nc.vector.tensor_scalar_mul(woi_rep[:], ones_kk[:], woi_t[:])

            # vu/vi broadcast across all 128 partitions: [128, H] PSUM
            vu_ps = psum.tile([HALF, H], f32, tag="vps")
            nc.tensor.matmul(vu_ps[:], wou_rep[:], wu_t[:], start=True, stop=True)
            vu_t = consts.tile([HALF, H], bf16)
            nc.vector.tensor_copy(vu_t[:], vu_ps[:])
            vi_ps = psum.tile([HALF, H], f32, tag="vps")
            nc.tensor.matmul(vi_ps[:], woi_rep[:], wi_t[:], start=True, stop=True)
            vi_t = consts.tile([HALF, H], bf16)
            nc.vector.tensor_copy(vi_t[:], vi_ps[:])

            # c = wo_u . b_user + wo_i . b_item + b_out, broadcast to [128,1]
            ones_k1 = consts.tile([HALF, 128], f32)
            nc.vector.memset(ones_k1[:], 1.0)
            cu_ps = psum.tile([128, 1], f32, tag="cps")
            bub = consts.tile([HALF, 128], f32)
            nc.vector.tensor_scalar_mul(bub[:], ones_k1[:], bu_t[:])
            bib = consts.tile([HALF, 128], f32)
            nc.vector.tensor_scalar_mul(bib[:], ones_k1[:], bi_t[:])
            nc.tensor.matmul(cu_ps[:], bub[:], wou_t[:], start=True, stop=False)
            nc.tensor.matmul(cu_ps[:], bib[:], woi_t[:], start=False, stop=False)
            nc.tensor.matmul(
                cu_ps[:], ones_k1[0:1, :], bo_t[:], start=False, stop=True
            )
            c_t = consts.tile([128, 1], f32)
            nc.vector.tensor_copy(c_t[:], cu_ps[:])

            # ---- item z phase first: feeds the scatters (issued before
            # the user gathers on the SWDGE queues) ----
            si_sb = spool.tile([128, I_TILES], f32)
            for ch in range(I_CHUNKS):
                zt = zpool.tile([128, CHUNK_T * H], bf16, tag="z")
                nc.sync.dma_start(
                    zt[:], zi.ap()[:, ch * CHUNK_T * H:(ch + 1) * CHUNK_T * H]
                )
                for t in range(CHUNK_T):
                    pr = scpool.tile([128, H], bf16, tag="scr")
                    nc.vector.tensor_mul(pr[:], zt[:, t * H:(t + 1) * H], vi_t[:])
                    nc.vector.reduce_sum(
                        si_sb[:, ch * CHUNK_T + t:ch * CHUNK_T + t + 1],
                        pr[:],
                        axis=mybir.AxisListType.X,
                    )

            # store item scores partition-major, reload as 4 scatter rows
            nc.sync.dma_start(
                s_ic.ap().rearrange("(p t) one -> p (t one)", p=128),
                si_sb[:],
            )
            sv_t = gpool.tile([NSC, SC_N], f32)
            nc.sync.dma_start(
                sv_t[:],
                s_ic.ap().rearrange("(a b) one -> a (b one)", a=NSC),
            )

            # ---- item expansion: scatter s_i to segment starts in dv ----
            for t in range(NSC):
                inst = nc.gpsimd.indirect_dma_start(
                    out=dv.ap(),
                    out_offset=IndirectOffsetOnAxis(
                        ap=idxsc_t[:, t * SCCOLS:(t + 1) * SCCOLS], axis=0
                    ),
                    in_=sv_t[t:t + 1, :].rearrange(
                        "one (c x) -> one c x", x=1
                    ),
                    in_offset=None,
                    bounds_check=E_CAP - 1,
                    oob_is_err=False,
                )
                inst.ins.queue = qnames[t % N_QUEUES]

            # ---- user z phase: its table gates the AllGather ----
            su_sb = spool.tile([128, U_TILES], f32)
            for ch in range(U_CHUNKS):
                zt = zpool.tile([128, CHUNK_T * H], bf16, tag="z")
                nc.sync.dma_start(
                    zt[:], zu.ap()[:, ch * CHUNK_T * H:(ch + 1) * CHUNK_T * H]
                )
                for t in range(CHUNK_T):
                    pr = scpool.tile([128, H], bf16, tag="scr")
                    nc.vector.tensor_mul(pr[:], zt[:, t * H:(t + 1) * H], vu_t[:])
                    nc.vector.reduce_sum(
                        su_sb[:, ch * CHUNK_T + t:ch * CHUNK_T + t + 1],
                        pr[:],
                        axis=mybir.AxisListType.X,
                    )
            # add folded constant c into the user table
            nc.vector.tensor_scalar_add(su_sb[:], su_sb[:], c_t[:])

            # store partition-major (one fat descriptor per partition) and
            # AllGather; the gather indices absorb the layout.
            nc.sync.dma_start(
                s_uc.ap().rearrange("(p t) one -> p (t one)", p=128),
                su_sb[:],
            )
            nc.gpsimd.collective_compute(
                "AllGather", mybir.AluOpType.bypass,
                replica_groups=groups, ins=[s_uc.ap()], outs=[s_uf.ap()],
            )

            # ---- user gathers, round-robin over the SWDGE queues ----
            gu_t = gpool.tile([128, RL], f32)
            for g in range(N_G):
                inst = nc.gpsimd.indirect_dma_start(
                    out=gu_t[7 * g:7 * g + 1, :].rearrange(
                        "one (c x) -> one c x", x=1
                    ),
                    out_offset=None,
                    in_=s_uf.ap(),
                    in_offset=IndirectOffsetOnAxis(
                        ap=idxu_t[:, g * GCOLS:(g + 1) * GCOLS], axis=0
                    ),
                )
                inst.ins.queue = qnames[g % N_QUEUES]

            # ---- compact gather rows 0,7,...,112 -> 17 partitions ----
            guc = gpool.tile([N_G, RL], f32, tag="dvg2")
            nc.sync.dma_start(guc[0:9, :], gu_t[0:57:7, :])
            nc.sync.dma_start(guc[9:N_G, :], gu_t[63:113:7, :])

            # ---- expansion scan: state = M*state + V per slot row ----
            dv_t = gpool.tile([N_G, RL], f32)
            nc.sync.dma_start(
                dv_t[:], dv.ap().rearrange("(a b) one -> a (b one)", a=N_G)
            )
            a_t = gpool.tile([N_G, RL], f32)
            nc.vector.tensor_tensor_scan(
                a_t[:], m_t[:], dv_t[:], 0.0,
                mybir.AluOpType.mult, mybir.AluOpType.add,
            )

            sc_t = gpool.tile([N_G, RL], f32, tag="zsc")
            nc.vector.tensor_add(sc_t[:], guc[:], a_t[:])
            nc.sync.dma_start(out.ap(), sc_t[:])

    nc.compile()
    _CACHE["nc"] = nc
    return nc


def _wrap_pf(vals, cols):
    """Partition-fastest wrap: stream element i -> tile[i%128, i//128]."""
    n = len(vals)
    assert n % 128 == 0 and n // 128 == cols
    return np.ascontiguousarray(vals.reshape(cols, 128).T)


def _swizzle_z(rows, tiles):
    """rows [n, H] -> bf16 [128, tiles*H], column block t = rows[t*128:(t+1)*128]."""
    import ml_dtypes

    n = rows.shape[0]
    padded = np.zeros((tiles * 128, H), dtype=np.float32)
    padded[:n] = rows
    return np.ascontiguousarray(
        padded.reshape(tiles, 128, H).transpose(1, 0, 2).reshape(128, tiles * H)
    ).astype(ml_dtypes.bfloat16)


def _uidx(u):
    """Global user id -> position in the partition-major AllGathered table."""
    c, n = u // U_SH, u % U_SH
    return c * U_PAD + (n % 128) * U_TILES + n // 128


def _pack_core(src_k, dst_k, base_item):
    """Pack one core's edges (sorted by dst) into N_G rows of RL slots,
    whole dst-segments per row.

    Returns (idxu [128, N_G*GCOLS], dest_stream [NSC*SC_N], mask [N_G, RL],
    eids [N_G, RL] position-in-src_k per slot or -1).
    """
    order = np.argsort(dst_k, kind="stable")
    dsts = dst_k[order]
    seg_starts = np.flatnonzero(np.r_[True, dsts[1:] != dsts[:-1]])
    seg_ends = np.r_[seg_starts[1:], len(dsts)]

    idxu_lin = np.zeros(E_CAP, dtype=np.int32)
    eids = np.full((N_G, RL), -1, dtype=np.int64)
    mask = np.ones((N_G, RL), dtype=np.float32)
    dest_node = np.full(I_SH, OOB, dtype=np.int32)

    row, pos = 0, 0
    for s, epos in zip(seg_starts, seg_ends):
        seg_len = epos - s
        if pos + seg_len > RL:
            row += 1
            pos = 0
            assert row < N_G, "edge packing overflow"
        item_local = dsts[s] - base_item
        slot0 = row * RL + pos
        dest_node[item_local] = slot0
        mask[row, pos] = 0.0
        eids[row, pos:pos + seg_len] = order[s:epos]
        idxu_lin[slot0:slot0 + seg_len] = _uidx(src_k[order[s:epos]])
        pos += seg_len

    idxu = np.empty((128, N_G * GCOLS), dtype=np.int32)
    for g in range(N_G):
        idxu[:, g * GCOLS:(g + 1) * GCOLS] = _wrap_pf(
            idxu_lin[g * RL:(g + 1) * RL], GCOLS
        )
    # scatter value stream j reads s_ic flat j = p*I_TILES + t, which holds
    # the score of local item n = 128*t + p
    j = np.arange(NSC * SC_N)
    n = 128 * (j % I_TILES) + j // I_TILES
    dest = np.where(n < I_SH, dest_node[np.minimum(n, I_SH - 1)], OOB)
    return idxu, dest.astype(np.int32), mask, eids


def _make_in_maps(inputs):
    z_user = np.asarray(inputs["z_user"], dtype=np.float32)
    z_item = np.asarray(inputs["z_item"], dtype=np.float32)
    src = np.asarray(inputs["edge_src"]).astype(np.int32)
    dst = np.asarray(inputs["edge_dst"]).astype(np.int32)
    w_user = np.asarray(inputs["w_user"], dtype=np.float32)
    w_item = np.asarray(inputs["w_item"], dtype=np.float32)
    b_user = np.asarray(inputs["b_user"], dtype=np.float32).reshape(HALF, 1)
    b_item = np.asarray(inputs["b_item"], dtype=np.float32).reshape(HALF, 1)
    w_out = np.asarray(inputs["w_out"], dtype=np.float32)
    b_out = np.asarray(inputs["b_out"], dtype=np.float32).reshape(1, 1)
    wo_u = w_out[0, :HALF].reshape(HALF, 1).copy()
    wo_i = w_out[0, HALF:].reshape(HALF, 1).copy()

    bucket = dst // I_SH
    in_maps = []
    slot_eids = []
    for k in range(N_CORES):
        sel = np.flatnonzero(bucket == k)
        idxu_m, dest, mask, eids = _pack_core(src[sel], dst[sel], k * I_SH)
        eids_g = np.where(eids >= 0, sel[np.clip(eids, 0, None)], -1)
        slot_eids.append(eids_g)
        idxsc_m = np.empty((128, NSC * SCCOLS), dtype=np.int32)
        for t in range(NSC):
            idxsc_m[:, t * SCCOLS:(t + 1) * SCCOLS] = _wrap_pf(
                dest[t * SC_N:(t + 1) * SC_N], SCCOLS
            )
        in_maps.append({
            "zu": _swizzle_z(z_user[k * U_SH:(k + 1) * U_SH], U_TILES),
            "zi": _swizzle_z(z_item[k * I_SH:(k + 1) * I_SH], I_TILES),
            "w_user": w_user,
            "w_item": w_item,
            "wo_u": wo_u,
            "wo_i": wo_i,
            "b_user": b_user,
            "b_item": b_item,
            "b_out": b_out,
            "idxu": idxu_m,
            "idxsc": idxsc_m,
            "mrow": mask,
        })
    return in_maps, slot_eids


def _run(inputs, trace=False):
    from concourse.bass_utils import run_bass_kernel_spmd

    nc = _build()
    in_maps, slot_eids = _make_in_maps(inputs)
    res = run_bass_kernel_spmd(
        nc, in_maps, core_ids=list(range(N_CORES)), trace=trace
    )
    full = np.empty(E, dtype=np.float32)
    for k in range(N_CORES):
        vals = res.results[k]["out"].reshape(-1)
        eids = slot_eids[k].reshape(-1)
        real = eids >= 0
        full[eids[real]] = vals[real]
    return full.reshape(E, 1), res


def kernel(**inputs):
    full, _ = _run(inputs, trace=False)
    return full



# revision 5
# speedup vs baseline: 3.1244x; 3.1244x over previous
"""Trainium2 Bass kernel for ContextAwareArtRecSys (gnn_message_passing).

Math fold: the reference is
    score[e] = concat(z_u[src] @ Wu.T + bu, z_i[dst] @ Wi.T + bi) @ wo.T + bo
Everything after the gather is linear, so with
    vu = wo[:, :128] @ Wu,  vi = wo[:, 128:] @ Wi          (256-vectors)
    c  = wo[:, :128]@bu + wo[:, 128:]@bi + bo              (scalar)
we have score[e] = (z_u @ vu)[src] + (z_i @ vi)[dst] + c.

v2 design — NO per-edge indirect-DMA descriptors (SWDGE service is
hard-serialized at ~4.2ns/descriptor, measured):

  1. Node scores z@v on DVE (fused tensor_tensor_reduce per 128-node
     tile), stored partition-major in bf16 and AllGathered (users and
     items separately).  Table position of a node is the wrapped order
     tpos = own*core + tiles*(n%128) + n//128 — host indices absorb it.
  2. Each core covers 62,500 edges in TWO row-bucketed layouts:
     space A rows = tposU(src)//392 (user side), space B rows =
     tposI(dst)//784 (item side), edges sorted within rows by tpos.
     Each row's node scores are a STATIC contiguous slice of the
     AllGathered table -> one plain DMA, no indirect descriptors.
  3. Per-row expansion: local_scatter (GPSIMD ucode, per-partition int16
     indices, ~4.3ns/idx) drops each node score at its first-edge slot,
     then a masked DVE scan (state = M*state + V) fills each segment.
  4. The item side is routed from space B to space A with
     local_scatter -> 16 PE tile-transposes -> local_scatter
     (any (q,j)->(p,f) permutation = row-permute, transpose, row-permute).
  5. One fused DVE op adds user + item + c; result DMAs out.
"""

import numpy as np

N_CORES = 8
N_USERS, N_ITEMS, E, H = 50000, 100000, 500000, 256
HALF = H // 2
EC = E // N_CORES                  # 62500 edges per core

U_OWN = 6272                       # users owned per core = 128*49
I_OWN = 12544                      # items owned per core = 128*98
UT = 49                            # user tiles per core
IT = 98                            # item tiles per core
U_PAD = 50176                      # 8*U_OWN = 128*392
I_PAD = 100352                     # 8*I_OWN = 128*784
UA = 392                           # user-table entries per A-row
IA = 784                           # item-table entries per B-row
F = 640                            # edge slots per row
CH = 7                             # z tiles per DMA chunk
STG = 2046                         # staging cols (local_scatter dst < 2048)
STG2 = 2048                        # post-transpose staging cols

_CACHE = {}


def _build():
    if "nc" in _CACHE:
        return _CACHE["nc"]
    import concourse.bass as bass
    import concourse.tile as tile
    import concourse.mybir as mybir
    from concourse import bacc

    f32 = mybir.dt.float32
    bf16 = mybir.dt.bfloat16
    i16 = mybir.dt.int16

    nc = bacc.Bacc("TRN2", target_bir_lowering=False, debug=False,
                   num_devices=N_CORES)

    zu = nc.dram_tensor("zu", [128, UT * H], bf16, kind="ExternalInput")
    zi = nc.dram_tensor("zi", [128, IT * H], bf16, kind="ExternalInput")
    w_user = nc.dram_tensor("w_user", [HALF, H], f32, kind="ExternalInput")
    w_item = nc.dram_tensor("w_item", [HALF, H], f32, kind="ExternalInput")
    wo_u = nc.dram_tensor("wo_u", [HALF, 1], f32, kind="ExternalInput")
    wo_i = nc.dram_tensor("wo_i", [HALF, 1], f32, kind="ExternalInput")
    b_user = nc.dram_tensor("b_user", [HALF, 1], f32, kind="ExternalInput")
    b_item = nc.dram_tensor("b_item", [HALF, 1], f32, kind="ExternalInput")
    b_out = nc.dram_tensor("b_out", [1, 1], f32, kind="ExternalInput")
    identd = nc.dram_tensor("identd", [128, 128], bf16, kind="ExternalInput")
    lsa = nc.dram_tensor("lsa", [128, UA], i16, kind="ExternalInput")
    lsb = nc.dram_tensor("lsb", [128, IA], i16, kind="ExternalInput")
    p1i = nc.dram_tensor("p1i", [128, F], i16, kind="ExternalInput")
    p2i = nc.dram_tensor("p2i", [128, STG2], i16, kind="ExternalInput")
    ma = nc.dram_tensor("ma", [128, F], bf16, kind="ExternalInput")
    mb = nc.dram_tensor("mb", [128, F], bf16, kind="ExternalInput")
    out = nc.dram_tensor("out", [128, F], f32, kind="ExternalOutput")

    su_loc = nc.dram_tensor("su_loc", [U_OWN, 1], bf16)
    si_loc = nc.dram_tensor("si_loc", [I_OWN, 1], bf16)
    su_all = nc.dram_tensor("su_all", [U_PAD, 1], bf16, addr_space="Shared")
    si_all = nc.dram_tensor("si_all", [I_PAD, 1], bf16, addr_space="Shared")

    groups = [list(range(N_CORES))]

    with tile.TileContext(nc) as tc:
        with (
            tc.tile_pool(name="consts", bufs=1) as consts,
            tc.tile_pool(name="zpool", bufs=3) as zpool,
            tc.tile_pool(name="scr", bufs=2) as scr,
            tc.tile_pool(name="work", bufs=1) as work,
            tc.tile_pool(name="psum", bufs=1, space="PSUM") as psum,
        ):
            # ---- weights for the fold ----
            wu_t = consts.tile([HALF, H], f32)
            nc.sync.dma_start(wu_t[:], w_user.ap())
            wi_t = consts.tile([HALF, H], f32)
            nc.sync.dma_start(wi_t[:], w_item.ap())
            wou_t = consts.tile([HALF, 1], f32)
            nc.sync.dma_start(wou_t[:], wo_u.ap())
            woi_t = consts.tile([HALF, 1], f32)
            nc.sync.dma_start(woi_t[:], wo_i.ap())
            bu_t = consts.tile([HALF, 1], f32)
            nc.sync.dma_start(bu_t[:], b_user.ap())
            bi_t = consts.tile([HALF, 1], f32)
            nc.sync.dma_start(bi_t[:], b_item.ap())
            bo_t = consts.tile([1, 1], f32)
            nc.sync.dma_start(bo_t[:], b_out.ap())

            # aux inputs for the expansion/routing phases
            ident = consts.tile([128, 128], bf16)
            nc.sync.dma_start(ident[:], identd.ap())
            lsa_t = consts.tile([128, UA], i16)
            nc.sync.dma_start(lsa_t[:], lsa.ap())
            lsb_t = consts.tile([128, IA], i16)
            nc.sync.dma_start(lsb_t[:], lsb.ap())
            p1_t = consts.tile([128, F], i16)
            nc.sync.dma_start(p1_t[:], p1i.ap())
            p2_t = consts.tile([128, STG2], i16)
            nc.sync.dma_start(p2_t[:], p2i.ap())
            ma_t = consts.tile([128, F], bf16)
            nc.sync.dma_start(ma_t[:], ma.ap())
            mb_t = consts.tile([128, F], bf16)
            nc.sync.dma_start(mb_t[:], mb.ap())

            # ---- fold vu / vi broadcast across partitions, and c ----
            ones_kk = consts.tile([HALF, HALF], f32)
            nc.vector.memset(ones_kk[:], 1.0)
            wou_rep = consts.tile([HALF, HALF], f32)
            nc.vector.tensor_scalar_mul(wou_rep[:], ones_kk[:], wou_t[:])
            woi_rep = consts.tile([HALF, HALF], f32)
            nc.vector.tensor_scalar_mul(woi_rep[:], ones_kk[:], woi_t[:])

            vu_ps = psum.tile([HALF, H], f32, tag="vps")
            nc.tensor.matmul(vu_ps[:], wou_rep[:], wu_t[:], start=True, stop=True)
            vu_t = consts.tile([HALF, H], bf16)
            nc.vector.tensor_copy(vu_t[:], vu_ps[:])
            vi_ps = psum.tile([HALF, H], f32, tag="vps")
            nc.tensor.matmul(vi_ps[:], woi_rep[:], wi_t[:], start=True, stop=True)
            vi_t = consts.tile([HALF, H], bf16)
            nc.vector.tensor_copy(vi_t[:], vi_ps[:])

            ones_k1 = consts.tile([HALF, 128], f32)
            nc.vector.memset(ones_k1[:], 1.0)
            bub = consts.tile([HALF, 128], f32)
            nc.vector.tensor_scalar_mul(bub[:], ones_k1[:], bu_t[:])
            bib = consts.tile([HALF, 128], f32)
            nc.vector.tensor_scalar_mul(bib[:], ones_k1[:], bi_t[:])
            cps = psum.tile([128, 1], f32, tag="cps")
            nc.tensor.matmul(cps[:], bub[:], wou_t[:], start=True, stop=False)
            nc.tensor.matmul(cps[:], bib[:], woi_t[:], start=False, stop=False)
            nc.tensor.matmul(cps[:], ones_k1[0:1, :], bo_t[:],
                             start=False, stop=True)
            c_t = consts.tile([128, 1], f32)
            nc.vector.tensor_copy(c_t[:], cps[:])

            # ---- user scores on DVE: fused (z*vu) + reduce per tile ----
            su_sb = work.tile([128, UT], f32)
            for ch in range(UT // CH):
                zch = zpool.tile([128, CH * H], bf16, tag="z")
                nc.sync.dma_start(
                    zch[:], zu.ap()[:, ch * CH * H:(ch + 1) * CH * H]
                )
                for t in range(CH):
                    pr = scr.tile([128, H], bf16, tag="pr")
                    col = ch * CH + t
                    nc.vector.tensor_mul(pr[:], zch[:, t * H:(t + 1) * H],
                                         vu_t[:])
                    nc.vector.reduce_sum(su_sb[:, col:col + 1], pr[:],
                                         axis=mybir.AxisListType.X)
            sc_u = work.tile([128, UT], bf16)
            nc.vector.tensor_copy(sc_u[:], su_sb[:])
            nc.sync.dma_start(
                su_loc.ap().rearrange("(p t) one -> p (t one)", p=128),
                sc_u[:],
            )
            nc.gpsimd.collective_compute(
                "AllGather", mybir.AluOpType.bypass,
                replica_groups=groups, ins=[su_loc.ap()], outs=[su_all.ap()],
            )

            # ---- user side: slice, seg-start scatter, scan ----
            usl = work.tile([128, UA], bf16)
            nc.sync.dma_start(
                usl[:], su_all.ap().rearrange("(p a) one -> p (a one)", p=128)
            )
            ua_t = work.tile([128, F], bf16)
            nc.gpsimd.local_scatter(ua_t[:], usl[:], lsa_t[:], 128, F, UA)
            uexp = work.tile([128, F], bf16)
            nc.vector.tensor_tensor_scan(
                uexp[:], ma_t[:], ua_t[:], 0.0,
                mybir.AluOpType.mult, mybir.AluOpType.add,
            )

            # ---- item scores on DVE ----
            si_sb = work.tile([128, IT], f32)
            for ch in range(IT // CH):
                zch = zpool.tile([128, CH * H], bf16, tag="z")
                nc.sync.dma_start(
                    zch[:], zi.ap()[:, ch * CH * H:(ch + 1) * CH * H]
                )
                for t in range(CH):
                    pr = scr.tile([128, H], bf16, tag="pr")
                    col = ch * CH + t
                    nc.vector.tensor_mul(pr[:], zch[:, t * H:(t + 1) * H],
                                         vi_t[:])
                    nc.vector.reduce_sum(si_sb[:, col:col + 1], pr[:],
                                         axis=mybir.AxisListType.X)
            sc_i = work.tile([128, IT], bf16)
            nc.vector.tensor_copy(sc_i[:], si_sb[:])
            nc.sync.dma_start(
                si_loc.ap().rearrange("(p t) one -> p (t one)", p=128),
                sc_i[:],
            )
            nc.gpsimd.collective_compute(
                "AllGather", mybir.AluOpType.bypass,
                replica_groups=groups, ins=[si_loc.ap()], outs=[si_all.ap()],
            )

            # ---- item side: slice, seg-start scatter, scan (space B) ----
            isl = work.tile([128, IA], bf16)
            nc.sync.dma_start(
                isl[:], si_all.ap().rearrange("(p a) one -> p (a one)", p=128)
            )
            ib_t = work.tile([128, F], bf16)
            nc.gpsimd.local_scatter(ib_t[:], isl[:], lsb_t[:], 128, F, IA)
            iexp = work.tile([128, F], bf16)
            nc.vector.tensor_tensor_scan(
                iexp[:], mb_t[:], ib_t[:], 0.0,
                mybir.AluOpType.mult, mybir.AluOpType.add,
            )

            # ---- route item scores B -> A: ls, transpose, ls ----
            stg = work.tile([128, STG], bf16)
            nc.gpsimd.local_scatter(stg[:], iexp[:], p1_t[:], 128, STG, F)
            pt = psum.tile([128, STG2], bf16, tag="pt")
            for t in range(15):
                nc.tensor.transpose(
                    pt[:, t * 128:(t + 1) * 128],
                    stg[:, t * 128:(t + 1) * 128], ident[:]
                )
            nc.tensor.transpose(
                pt[0:STG - 1920, 1920:2048], stg[:, 1920:STG], ident[:]
            )
            stg2 = work.tile([128, STG2], bf16)
            nc.vector.tensor_copy(stg2[:], pt[:])
            iex2 = work.tile([128, F], bf16)
            nc.gpsimd.local_scatter(iex2[:], stg2[:], p2_t[:], 128, F, STG2)

            # ---- combine: out = (uexp + c) + iex2 ----
            outf = work.tile([128, F], f32)
            nc.vector.scalar_tensor_tensor(
                outf[:], uexp[:], c_t[:], iex2[:],
                mybir.AluOpType.add, mybir.AluOpType.add,
            )
            nc.sync.dma_start(out.ap(), outf[:])

    nc.compile()
    _CACHE["nc"] = nc
    return nc


def _swizzle_z(rows, tiles):
    """rows [n, H] -> bf16 [128, tiles*H]; tile t cols = rows[t*128:(t+1)*128]."""
    import ml_dtypes

    n = rows.shape[0]
    padded = np.zeros((tiles * 128, H), dtype=np.float32)
    padded[:n] = rows
    return np.ascontiguousarray(
        padded.reshape(tiles, 128, H).transpose(1, 0, 2).reshape(128, tiles * H)
    ).astype(ml_dtypes.bfloat16)


def _tpos_u(u):
    n = u % U_OWN
    return U_OWN * (u // U_OWN) + UT * (n % 128) + n // 128


def _tpos_i(d):
    n = d % I_OWN
    return I_OWN * (d // I_OWN) + IT * (n % 128) + n // 128


def _rank_in_group(keys):
    order = np.argsort(keys, kind="stable")
    ks = keys[order]
    first = np.r_[True, ks[1:] != ks[:-1]]
    gstart = np.where(first)[0]
    ranks_sorted = np.arange(len(keys)) - np.repeat(
        gstart, np.diff(np.r_[gstart, len(keys)])
    )
    ranks = np.empty(len(keys), dtype=np.int64)
    ranks[order] = ranks_sorted
    return ranks


def _pack_core(es, ed):
    """Host-side index construction for one core's edges.

    es/ed are TABLE POSITIONS (tpos) of each edge's endpoints."""
    orderA = np.argsort(es, kind="stable")
    sa = es[orderA]
    paS = sa // UA
    rowstartA = np.searchsorted(paS, np.arange(128))
    jaS = np.arange(EC) - rowstartA[paS]
    assert jaS.max() < F, f"A row overflow {jaS.max()}"
    firstA = np.r_[True, sa[1:] != sa[:-1]]
    lsa_m = np.full((128, UA), -1, dtype=np.int16)
    lsa_m[paS[firstA], sa[firstA] % UA] = jaS[firstA]
    ma_m = np.ones((128, F), dtype=np.float32)
    ma_m[paS[firstA], jaS[firstA]] = 0.0

    paE = es // UA
    posA = np.empty(EC, dtype=np.int64)
    posA[orderA] = np.arange(EC)
    jaE = posA - rowstartA[paE]

    orderB = np.argsort(ed, kind="stable")
    db = ed[orderB]
    qbS = db // IA
    rowstartB = np.searchsorted(qbS, np.arange(128))
    jbS = np.arange(EC) - rowstartB[qbS]
    assert jbS.max() < F, f"B row overflow {jbS.max()}"
    firstB = np.r_[True, db[1:] != db[:-1]]
    lsb_m = np.full((128, IA), -1, dtype=np.int16)
    lsb_m[qbS[firstB], db[firstB] % IA] = jbS[firstB]
    mb_m = np.ones((128, F), dtype=np.float32)
    mb_m[qbS[firstB], jbS[firstB]] = 0.0

    qbE = ed // IA
    posB = np.empty(EC, dtype=np.int64)
    posB[orderB] = np.arange(EC)
    jbE = posB - rowstartB[qbE]

    # routing: staging col = 128*t + paE on row qbE; after transpose the
    # value sits at (paE, 128*t + qbE) and moves to final slot jaE.
    tE = _rank_in_group(qbE * 128 + paE)
    scol = 128 * tE + paE
    assert scol.max() < STG, f"staging overflow {scol.max()}"
    p1_m = np.full((128, F), -1, dtype=np.int16)
    p1_m[qbE, jbE] = scol
    p2_m = np.full((128, STG2), -1, dtype=np.int16)
    p2_m[paE, 128 * tE + qbE] = jaE

    return {
        "lsa": lsa_m, "lsb": lsb_m, "p1i": p1_m, "p2i": p2_m,
        "ma": ma_m, "mb": mb_m, "paE": paE, "jaE": jaE,
    }


def _make_in_maps(inputs):
    import ml_dtypes

    z_user = np.asarray(inputs["z_user"], dtype=np.float32)
    z_item = np.asarray(inputs["z_item"], dtype=np.float32)
    src = np.asarray(inputs["edge_src"]).astype(np.int64)
    dst = np.asarray(inputs["edge_dst"]).astype(np.int64)
    w_user = np.asarray(inputs["w_user"], dtype=np.float32)
    w_item = np.asarray(inputs["w_item"], dtype=np.float32)
    b_user = np.asarray(inputs["b_user"], dtype=np.float32).reshape(HALF, 1)
    b_item = np.asarray(inputs["b_item"], dtype=np.float32).reshape(HALF, 1)
    w_out = np.asarray(inputs["w_out"], dtype=np.float32)
    b_out = np.asarray(inputs["b_out"], dtype=np.float32).reshape(1, 1)
    wo_u = w_out[0, :HALF].reshape(HALF, 1).copy()
    wo_i = w_out[0, HALF:].reshape(HALF, 1).copy()
    ident = np.eye(128, dtype=np.float32).astype(ml_dtypes.bfloat16)

    tpu = _tpos_u(src)
    tpi = _tpos_i(dst)

    in_maps, metas = [], []
    for k in range(N_CORES):
        zu_sh = z_user[k * U_OWN:min((k + 1) * U_OWN, N_USERS)]
        zi_sh = z_item[k * I_OWN:min((k + 1) * I_OWN, N_ITEMS)]
        m = _pack_core(tpu[k * EC:(k + 1) * EC], tpi[k * EC:(k + 1) * EC])
        metas.append(m)
        in_maps.append({
            "zu": _swizzle_z(zu_sh, UT),
            "zi": _swizzle_z(zi_sh, IT),
            "w_user": w_user, "w_item": w_item,
            "wo_u": wo_u, "wo_i": wo_i,
            "b_user": b_user, "b_item": b_item, "b_out": b_out,
            "identd": ident,
            "lsa": m["lsa"], "lsb": m["lsb"],
            "p1i": m["p1i"], "p2i": m["p2i"],
            "ma": m["ma"].astype(ml_dtypes.bfloat16),
            "mb": m["mb"].astype(ml_dtypes.bfloat16),
        })
    return in_maps, metas


def _run(inputs, trace=False):
    from concourse.bass_utils import run_bass_kernel_spmd

    nc = _build()
    in_maps, metas = _make_in_maps(inputs)
    res = run_bass_kernel_spmd(
        nc, in_maps, core_ids=list(range(N_CORES)), trace=trace
    )
    full = np.empty(E, dtype=np.float32)
    for k in range(N_CORES):
        o = res.results[k]["out"]
        m = metas[k]
        full[k * EC:(k + 1) * EC] = o[m["paE"], m["jaE"]]
    return full.reshape(E, 1), res


def kernel(**inputs):
    full, _ = _run(inputs, trace=False)
    return full


# revision 7
# speedup vs baseline: 3.3510x; 1.0725x over previous
"""Trainium2 Bass kernel for ContextAwareArtRecSys (gnn_message_passing).

Math fold: the reference is
    score[e] = concat(z_u[src] @ Wu.T + bu, z_i[dst] @ Wi.T + bi) @ wo.T + bo
Everything after the gather is linear, so with
    vu = wo[:, :128] @ Wu,  vi = wo[:, 128:] @ Wi          (256-vectors)
    c  = wo[:, :128]@bu + wo[:, 128:]@bi + bo              (scalar)
we have score[e] = (z_u @ vu)[src] + (z_i @ vi)[dst] + c.

v3 design — no per-edge indirect-DMA descriptors (SWDGE service is
hard-serialized at ~4.2ns/descriptor, measured):

  1. Node scores z@v on the PE: z is shipped transposed in 512-node
     chunks, v is a 1-column stationary, each chunk accumulates h-halves
     into a [1,512] PSUM row which the Activation engine drains to a
     bf16 score row in SBUF.  One combined AllGather publishes every
     core's (users | items) score block.
  2. Each core covers 62,500 edges in TWO row-bucketed layouts:
     space A rows = user-table position // 392 (user side), space B
     rows = item-table position // 784.  Each row's node scores are a
     STATIC contiguous slice of the AllGathered table -> plain DMAs.
  3. Per-row expansion: local_scatter (GPSIMD ucode, per-partition int16
     indices, ~4.3ns/idx) drops each node score at its first-edge slot,
     then a masked DVE scan (state = M*state + V) fills each segment.
  4. The item side is routed from space B to space A with
     local_scatter -> 16 PE tile-transposes -> local_scatter
     (any (q,j)->(p,f) permutation = row-permute, transpose, row-permute).
  5. One fused DVE op adds user + item + c; result DMAs out.
"""

import numpy as np

N_CORES = 8
N_USERS, N_ITEMS, E, H = 50000, 100000, 500000, 256
HALF = H // 2
EC = E // N_CORES                  # 62500 edges per core

U_OWN = 6272                       # users owned per core (8*6272 = 50176)
I_OWN = 12544                      # items owned per core (8*12544 = 100352)
S_OWN = U_OWN + I_OWN              # 18816 per-core score block
UCH = 13                           # user psum chunks of 512 (6656 slots)
ICH = 25                           # item psum chunks of 512 (12800 slots)
TCH = UCH + ICH                    # 38
USLOT = UCH * 512                  # 6656
UA = 392                           # user-table entries per A-row (16/core)
IA = 784                           # item-table entries per B-row (16/core)
F = 640                            # edge slots per row
STG = 2046                         # staging cols (local_scatter dst < 2048)
STG2 = 2048                        # post-transpose staging cols

_CACHE = {}


def _build():
    if "nc" in _CACHE:
        return _CACHE["nc"]
    import concourse.bass as bass
    import concourse.tile as tile
    import concourse.mybir as mybir
    from concourse import bacc

    f32 = mybir.dt.float32
    bf16 = mybir.dt.bfloat16
    i16 = mybir.dt.int16

    nc = bacc.Bacc("TRN2", target_bir_lowering=False, debug=False,
                   num_devices=N_CORES)

    zt = nc.dram_tensor("zt", [128, TCH * 1024], bf16, kind="ExternalInput")
    w_user = nc.dram_tensor("w_user", [HALF, H], f32, kind="ExternalInput")
    w_item = nc.dram_tensor("w_item", [HALF, H], f32, kind="ExternalInput")
    wo_u = nc.dram_tensor("wo_u", [HALF, 1], f32, kind="ExternalInput")
    wo_i = nc.dram_tensor("wo_i", [HALF, 1], f32, kind="ExternalInput")
    b_user = nc.dram_tensor("b_user", [HALF, 1], f32, kind="ExternalInput")
    b_item = nc.dram_tensor("b_item", [HALF, 1], f32, kind="ExternalInput")
    b_out = nc.dram_tensor("b_out", [1, 1], f32, kind="ExternalInput")
    identd = nc.dram_tensor("identd", [128, 128], bf16, kind="ExternalInput")
    lsa = nc.dram_tensor("lsa", [128, UA], i16, kind="ExternalInput")
    lsb = nc.dram_tensor("lsb", [128, IA], i16, kind="ExternalInput")
    p1i = nc.dram_tensor("p1i", [128, F], i16, kind="ExternalInput")
    p2i = nc.dram_tensor("p2i", [128, STG2], i16, kind="ExternalInput")
    ma = nc.dram_tensor("ma", [128, F], bf16, kind="ExternalInput")
    mb = nc.dram_tensor("mb", [128, F], bf16, kind="ExternalInput")
    out = nc.dram_tensor("out", [128, F], f32, kind="ExternalOutput")

    s_loc = nc.dram_tensor("s_loc", [S_OWN, 1], bf16)
    s_all = nc.dram_tensor("s_all", [N_CORES * S_OWN, 1], bf16,
                           addr_space="Shared")

    groups = [list(range(N_CORES))]

    with tile.TileContext(nc) as tc:
        with (
            tc.tile_pool(name="consts", bufs=1) as consts,
            tc.tile_pool(name="zpool", bufs=3) as zpool,
            tc.tile_pool(name="work", bufs=1) as work,
            tc.tile_pool(name="psum", bufs=1, space="PSUM") as psum,
            tc.tile_pool(name="spsum", bufs=4, space="PSUM") as spsum,
        ):
            # ---- weights for the fold ----
            wu_t = consts.tile([HALF, H], f32)
            nc.sync.dma_start(wu_t[:], w_user.ap())
            wi_t = consts.tile([HALF, H], f32)
            nc.sync.dma_start(wi_t[:], w_item.ap())
            wou_t = consts.tile([HALF, 1], f32)
            nc.sync.dma_start(wou_t[:], wo_u.ap())
            woi_t = consts.tile([HALF, 1], f32)
            nc.sync.dma_start(woi_t[:], wo_i.ap())
            bu_t = consts.tile([HALF, 1], f32)
            nc.sync.dma_start(bu_t[:], b_user.ap())
            bi_t = consts.tile([HALF, 1], f32)
            nc.sync.dma_start(bi_t[:], b_item.ap())
            bo_t = consts.tile([1, 1], f32)
            nc.sync.dma_start(bo_t[:], b_out.ap())

            # aux inputs for the expansion/routing phases
            ident = consts.tile([128, 128], bf16)
            nc.sync.dma_start(ident[:], identd.ap())
            lsa_t = consts.tile([128, UA], i16)
            nc.sync.dma_start(lsa_t[:], lsa.ap())
            lsb_t = consts.tile([128, IA], i16)
            nc.sync.dma_start(lsb_t[:], lsb.ap())
            p1_t = consts.tile([128, F], i16)
            nc.sync.dma_start(p1_t[:], p1i.ap())
            p2_t = consts.tile([128, STG2], i16)
            nc.sync.dma_start(p2_t[:], p2i.ap())
            ma_t = consts.tile([128, F], bf16)
            nc.sync.dma_start(ma_t[:], ma.ap())
            mb_t = consts.tile([128, F], bf16)
            nc.sync.dma_start(mb_t[:], mb.ap())

            # ---- fold: vT columns (vu0 | vu1 | vi0 | vi1), and c ----
            vps = psum.tile([128, 4], f32, tag="vps")
            nc.tensor.matmul(vps[:, 0:1], wu_t[:, 0:HALF], wou_t[:],
                             start=True, stop=True)
            nc.tensor.matmul(vps[:, 1:2], wu_t[:, HALF:H], wou_t[:],
                             start=True, stop=True)
            nc.tensor.matmul(vps[:, 2:3], wi_t[:, 0:HALF], woi_t[:],
                             start=True, stop=True)
            nc.tensor.matmul(vps[:, 3:4], wi_t[:, HALF:H], woi_t[:],
                             start=True, stop=True)
            vT = consts.tile([128, 4], bf16)
            nc.vector.tensor_copy(vT[:], vps[:])

            ones_k1 = consts.tile([HALF, 128], f32)
            nc.vector.memset(ones_k1[:], 1.0)
            bub = consts.tile([HALF, 128], f32)
            nc.vector.tensor_scalar_mul(bub[:], ones_k1[:], bu_t[:])
            bib = consts.tile([HALF, 128], f32)
            nc.vector.tensor_scalar_mul(bib[:], ones_k1[:], bi_t[:])
            cps = psum.tile([128, 1], f32, tag="cps")
            nc.tensor.matmul(cps[:], bub[:], wou_t[:], start=True, stop=False)
            nc.tensor.matmul(cps[:], bib[:], woi_t[:], start=False, stop=False)
            nc.tensor.matmul(cps[:], ones_k1[0:1, :], bo_t[:],
                             start=False, stop=True)
            c_t = consts.tile([128, 1], f32)
            nc.vector.tensor_copy(c_t[:], cps[:])

            # ---- node scores on PE, drained by Activation to bf16 row ----
            srow = work.tile([1, TCH * 512], bf16)
            for c in range(TCH):
                zch = zpool.tile([128, 1024], bf16, tag="z")
                nc.sync.dma_start(
                    zch[:], zt.ap()[:, c * 1024:(c + 1) * 1024]
                )
                a = 0 if c < UCH else 2
                ps = spsum.tile([1, 512], f32, tag="sc")
                nc.tensor.matmul(ps[:], vT[:, a:a + 1], zch[:, 0:512],
                                 start=True, stop=False)
                nc.tensor.matmul(ps[:], vT[:, a + 1:a + 2], zch[:, 512:1024],
                                 start=False, stop=True)
                nc.scalar.copy(srow[0:1, c * 512:(c + 1) * 512], ps[:])

            nc.sync.dma_start(
                s_loc.ap()[0:U_OWN].rearrange("(a b) one -> a (b one)", a=1),
                srow[0:1, 0:U_OWN],
            )
            nc.sync.dma_start(
                s_loc.ap()[U_OWN:S_OWN].rearrange("(a b) one -> a (b one)", a=1),
                srow[0:1, USLOT:USLOT + I_OWN],
            )
            nc.gpsimd.collective_compute(
                "AllGather", mybir.AluOpType.bypass,
                replica_groups=groups, ins=[s_loc.ap()], outs=[s_all.ap()],
            )

            # ---- table slices: A rows (users), B rows (items) ----
            # row (k, r): users = s_all[S_OWN*k + 392*r + j]
            usl = work.tile([128, UA], bf16)
            isl = work.tile([128, IA], bf16)
            for k in range(N_CORES):
                nc.sync.dma_start(
                    usl[16 * k:16 * (k + 1), :],
                    s_all.ap()[S_OWN * k:S_OWN * k + U_OWN].rearrange(
                        "(r a) one -> r (a one)", r=16),
                )
                nc.sync.dma_start(
                    isl[16 * k:16 * (k + 1), :],
                    s_all.ap()[S_OWN * k + U_OWN:S_OWN * (k + 1)].rearrange(
                        "(r a) one -> r (a one)", r=16),
                )

            # ---- user side: seg-start scatter + scan ----
            ua_t = work.tile([128, F], bf16)
            nc.gpsimd.local_scatter(ua_t[:], usl[:], lsa_t[:], 128, F, UA)
            uexp = work.tile([128, F], bf16)
            nc.vector.tensor_tensor_scan(
                uexp[:], ma_t[:], ua_t[:], 0.0,
                mybir.AluOpType.mult, mybir.AluOpType.add,
            )

            # ---- item side: seg-start scatter + scan (space B) ----
            ib_t = work.tile([128, F], bf16)
            nc.gpsimd.local_scatter(ib_t[:], isl[:], lsb_t[:], 128, F, IA)
            iexp = work.tile([128, F], bf16)
            nc.vector.tensor_tensor_scan(
                iexp[:], mb_t[:], ib_t[:], 0.0,
                mybir.AluOpType.mult, mybir.AluOpType.add,
            )

            # ---- route item scores B -> A: ls, transpose, ls ----
            stg = work.tile([128, STG], bf16)
            nc.gpsimd.local_scatter(stg[:], iexp[:], p1_t[:], 128, STG, F)
            pt = psum.tile([128, STG2], bf16, tag="pt")
            for t in range(15):
                nc.tensor.transpose(
                    pt[:, t * 128:(t + 1) * 128],
                    stg[:, t * 128:(t + 1) * 128], ident[:]
                )
            nc.tensor.transpose(
                pt[0:STG - 1920, 1920:2048], stg[:, 1920:STG], ident[:]
            )
            stg2 = work.tile([128, STG2], bf16)
            nc.vector.tensor_copy(stg2[:], pt[:])
            iex2 = work.tile([128, F], bf16)
            nc.gpsimd.local_scatter(iex2[:], stg2[:], p2_t[:], 128, F, STG2)

            # ---- combine: out = (uexp + c) + iex2 ----
            outf = work.tile([128, F], f32)
            nc.vector.scalar_tensor_tensor(
                outf[:], uexp[:], c_t[:], iex2[:],
                mybir.AluOpType.add, mybir.AluOpType.add,
            )
            nc.sync.dma_start(out.ap(), outf[:])

    nc.compile()
    _CACHE["nc"] = nc
    return nc


def _swizzle_zt(zu_sh, zi_sh):
    """[128, TCH*1024] bf16: chunk c = 512-node block; cols = h0|h1 halves.
    zT[h, 1024c + half*512 + j] = z[node(512c + j), half*128 + h]."""
    import ml_dtypes

    zu = np.zeros((UCH * 512, H), dtype=np.float32)
    zu[:zu_sh.shape[0]] = zu_sh
    zi = np.zeros((ICH * 512, H), dtype=np.float32)
    zi[:zi_sh.shape[0]] = zi_sh
    zall = np.concatenate([zu, zi], axis=0)            # [TCH*512, 256]
    zr = zall.reshape(TCH, 512, 2, 128).transpose(3, 0, 2, 1)
    return np.ascontiguousarray(
        zr.reshape(128, TCH * 1024)
    ).astype(ml_dtypes.bfloat16)


def _tpos_u(u):
    return S_OWN * (u // U_OWN) + u % U_OWN


def _tpos_i(d):
    return S_OWN * (d // I_OWN) + U_OWN + d % I_OWN


def _rank_in_group(keys):
    order = np.argsort(keys, kind="stable")
    ks = keys[order]
    first = np.r_[True, ks[1:] != ks[:-1]]
    gstart = np.where(first)[0]
    ranks_sorted = np.arange(len(keys)) - np.repeat(
        gstart, np.diff(np.r_[gstart, len(keys)])
    )
    ranks = np.empty(len(keys), dtype=np.int64)
    ranks[order] = ranks_sorted
    return ranks


def _pack_core2(tpu, tpi):
    """Host-side index construction for one core's edges, from the
    table positions (tpu/tpi) of each edge's endpoints."""
    # user side rows: tpos = S_OWN*k + local, local < U_OWN
    ka = tpu // S_OWN
    la = tpu % S_OWN
    paE = 16 * ka + la // UA
    oaE = la % UA                                # offset within row slice
    keyA = paE * (UA + 1) + oaE                  # sort key: (row, offset)
    orderA = np.argsort(keyA, kind="stable")
    paS = paE[orderA]
    oaS = oaE[orderA]
    rowstartA = np.searchsorted(paS, np.arange(128))
    jaS = np.arange(EC) - rowstartA[paS]
    assert jaS.max() < F, f"A row overflow {jaS.max()}"
    firstA = np.r_[True, (paS[1:] != paS[:-1]) | (oaS[1:] != oaS[:-1])]
    lsa_m = np.full((128, UA), -1, dtype=np.int16)
    lsa_m[paS[firstA], oaS[firstA]] = jaS[firstA]
    ma_m = np.ones((128, F), dtype=np.float32)
    ma_m[paS[firstA], jaS[firstA]] = 0.0

    posA = np.empty(EC, dtype=np.int64)
    posA[orderA] = np.arange(EC)
    jaE = posA - rowstartA[paE]

    # item side rows
    kb = tpi // S_OWN
    lb = tpi % S_OWN - U_OWN
    qbE = 16 * kb + lb // IA
    obE = lb % IA
    keyB = qbE * (IA + 1) + obE
    orderB = np.argsort(keyB, kind="stable")
    qbS = qbE[orderB]
    obS = obE[orderB]
    rowstartB = np.searchsorted(qbS, np.arange(128))
    jbS = np.arange(EC) - rowstartB[qbS]
    assert jbS.max() < F, f"B row overflow {jbS.max()}"
    firstB = np.r_[True, (qbS[1:] != qbS[:-1]) | (obS[1:] != obS[:-1])]
    lsb_m = np.full((128, IA), -1, dtype=np.int16)
    lsb_m[qbS[firstB], obS[firstB]] = jbS[firstB]
    mb_m = np.ones((128, F), dtype=np.float32)
    mb_m[qbS[firstB], jbS[firstB]] = 0.0

    posB = np.empty(EC, dtype=np.int64)
    posB[orderB] = np.arange(EC)
    jbE = posB - rowstartB[qbE]

    # routing: staging col = 128*t + paE on row qbE; after transpose the
    # value sits at (paE, 128*t + qbE) and moves to final slot jaE.
    tE = _rank_in_group(qbE * 128 + paE)
    scol = 128 * tE + paE
    assert scol.max() < STG, f"staging overflow {scol.max()}"
    p1_m = np.full((128, F), -1, dtype=np.int16)
    p1_m[qbE, jbE] = scol
    p2_m = np.full((128, STG2), -1, dtype=np.int16)
    p2_m[paE, 128 * tE + qbE] = jaE

    return {
        "lsa": lsa_m, "lsb": lsb_m, "p1i": p1_m, "p2i": p2_m,
        "ma": ma_m, "mb": mb_m, "paE": paE, "jaE": jaE,
    }


def _make_in_maps(inputs):
    import ml_dtypes

    z_user = np.asarray(inputs["z_user"], dtype=np.float32)
    z_item = np.asarray(inputs["z_item"], dtype=np.float32)
    src = np.asarray(inputs["edge_src"]).astype(np.int64)
    dst = np.asarray(inputs["edge_dst"]).astype(np.int64)
    w_user = np.asarray(inputs["w_user"], dtype=np.float32)
    w_item = np.asarray(inputs["w_item"], dtype=np.float32)
    b_user = np.asarray(inputs["b_user"], dtype=np.float32).reshape(HALF, 1)
    b_item = np.asarray(inputs["b_item"], dtype=np.float32).reshape(HALF, 1)
    w_out = np.asarray(inputs["w_out"], dtype=np.float32)
    b_out = np.asarray(inputs["b_out"], dtype=np.float32).reshape(1, 1)
    wo_u = w_out[0, :HALF].reshape(HALF, 1).copy()
    wo_i = w_out[0, HALF:].reshape(HALF, 1).copy()
    ident = np.eye(128, dtype=np.float32).astype(ml_dtypes.bfloat16)

    tpu = _tpos_u(src)
    tpi = _tpos_i(dst)

    in_maps, metas = [], []
    for k in range(N_CORES):
        zu_sh = z_user[k * U_OWN:min((k + 1) * U_OWN, N_USERS)]
        zi_sh = z_item[k * I_OWN:min((k + 1) * I_OWN, N_ITEMS)]
        m = _pack_core2(tpu[k * EC:(k + 1) * EC], tpi[k * EC:(k + 1) * EC])
        metas.append(m)
        in_maps.append({
            "zt": _swizzle_zt(zu_sh, zi_sh),
            "w_user": w_user, "w_item": w_item,
            "wo_u": wo_u, "wo_i": wo_i,
            "b_user": b_user, "b_item": b_item, "b_out": b_out,
            "identd": ident,
            "lsa": m["lsa"], "lsb": m["lsb"],
            "p1i": m["p1i"], "p2i": m["p2i"],
            "ma": m["ma"].astype(ml_dtypes.bfloat16),
            "mb": m["mb"].astype(ml_dtypes.bfloat16),
        })
    return in_maps, metas


def _run(inputs, trace=False):
    from concourse.bass_utils import run_bass_kernel_spmd

    nc = _build()
    in_maps, metas = _make_in_maps(inputs)
    res = run_bass_kernel_spmd(
        nc, in_maps, core_ids=list(range(N_CORES)), trace=trace
    )
    full = np.empty(E, dtype=np.float32)
    for k in range(N_CORES):
        o = res.results[k]["out"]
        m = metas[k]
        full[k * EC:(k + 1) * EC] = o[m["paE"], m["jaE"]]
    return full.reshape(E, 1), res


def kernel(**inputs):
    full, _ = _run(inputs, trace=False)
    return full
